# revision 65
# baseline (speedup 1.0000x reference)
"""Trainium2 Bass kernel for nn_MessagePassingNet (SAGEConv + TopKPooling net).

Contract: kernel(**inputs) takes the FULL unsharded inputs (as produced by
setup_inputs()) and returns the FULL [512, 8] output. Internally the 512
graphs are sharded contiguously across 8 NeuronCores (64 graphs each); the
small weights are replicated. All graph compute (adjacency build from the
edge list, 3x SAGE conv, 3x top-k pooling, readout MLP) runs on-device via
a Bass/Tile kernel; the host only slices inputs per core and reassembles
the per-core outputs.

v3: packed-source adjacency build. Nodes are permuted per graph by parity
(pos(n) = n>>1 + 128*(n&1)) so the source one-hot packs two nodes per
column: cs[e,j] = [src>>1==j] * (1 + 511*(src&1)) is built in ONE 128-col
tensor_scalar (is_equal then mult by a per-edge weight in {1,512}).
One [128,256] matmul per 128-edge chunk accumulates P[j,d] = A_even[j,d]
+ 512*A_odd[j,d]; after all chunks, hi = int32((P - 255.75)/512) (exact by
round-to-nearest), lo = P - 512*hi unpack the two source-parity planes of
the adjacency directly into the permuted-row layout of A_all. Chunk work is
split across DVE (fp16 packed), Pool (fp8e5, DoubleRow matmul pairs), and
Act (|iota-d| + relu(1-t) two-pass one-hots). Phase 2 (convs, top-k pools,
MLP) unchanged from v2: adjacency resident in SBUF, mean-division on Act,
feature transposes via DMA.
"""
import sys

sys.path.insert(0, "/opt/trn_rl_repo")

import os
import numpy as np
import ml_dtypes

import concourse.bacc as bacc
import concourse.mybir as mybir
from concourse.tile import TileContext
from concourse import bass_utils

dt = mybir.dt
Alu = mybir.AluOpType
Act = mybir.ActivationFunctionType
PerfMode = mybir.MatmulPerfMode

PHASES = int(os.environ.get("GNN_PHASES", "9"))
B, NPG, EPG, F, H, T = 512, 256, 4096, 128, 128, 8
N_CORES = 8
G = B // N_CORES          # 64 graphs per core
K1, K2, K3 = 205, 164, 132
KS = [K1, K2, K3]
DROPS = [256 - K1, K1 - K2, K2 - K3]
# adjacency chunk split per graph (32 chunks of 128 edges each):
POOL_PAIRS = int(os.environ.get("GNN_POOL_PAIRS", "3"))   # fp8e5 DR pairs
ACT_CHUNKS = int(os.environ.get("GNN_ACT_CHUNKS", "4"))   # dst one-hot on Act
POOL_DST = int(os.environ.get("GNN_POOL_DST", "2"))       # dst one-hot on Pool
DVE_CHUNKS = 32 - 2 * POOL_PAIRS - ACT_CHUNKS - POOL_DST


def build_gnn(nc, tc):
    NT = 2 * G
    NN = G * NPG
    NE = G * EPG
    EPC = NE // 128

    f32, fp16, fp8, i32 = dt.float32, dt.float16, dt.float8e5, dt.int32

    xh = nc.dram_tensor("xh", [NN, F], fp16, kind="ExternalInput")
    srch_d = nc.dram_tensor("srch", [128, EPC], f32, kind="ExternalInput")
    srcw_d = nc.dram_tensor("srcw", [128, EPC], f32, kind="ExternalInput")
    dstp_d = nc.dram_tensor("dstp", [128, EPC], f32, kind="ExternalInput")
    wl = [nc.dram_tensor(f"w{k}l", [F, H], fp16, kind="ExternalInput") for k in range(3)]
    wr = [nc.dram_tensor(f"w{k}r", [F, H], fp16, kind="ExternalInput") for k in range(3)]
    bias = [nc.dram_tensor(f"b{k}", [H, 1], f32, kind="ExternalInput") for k in range(3)]
    wcol_d = [nc.dram_tensor(f"wcol{k}", [128, 1], fp16, kind="ExternalInput") for k in range(3)]
    iota_d = nc.dram_tensor("iota256", [128, 256], fp16, kind="ExternalInput")
    ident_d = nc.dram_tensor("ident", [128, 128], f32, kind="ExternalInput")
    eu_d = nc.dram_tensor("eu", [NT, 2 * G], f32, kind="ExternalInput")
    fu_d = nc.dram_tensor("fu", [G, 2 * NT], f32, kind="ExternalInput")
    l1wa = nc.dram_tensor("l1wa", [128, 128], f32, kind="ExternalInput")
    l1wb = nc.dram_tensor("l1wb", [128, 128], f32, kind="ExternalInput")
    l2w = nc.dram_tensor("l2w", [128, 64], f32, kind="ExternalInput")
    l3w = nc.dram_tensor("l3w", [64, T], f32, kind="ExternalInput")
    l1b = nc.dram_tensor("l1b", [128, 1], f32, kind="ExternalInput")
    l2b = nc.dram_tensor("l2b", [64, 1], f32, kind="ExternalInput")
    l3b = nc.dram_tensor("l3b", [T, 1], f32, kind="ExternalInput")
    out_d = nc.dram_tensor("out", [G, T], f32, kind="ExternalOutput")

    BUF = [nc.alloc_sbuf_tensor(f"big{i}", [128, NT * 128], fp16) for i in range(4)]
    A_all = nc.alloc_sbuf_tensor("A_all", [128, G * 512], fp16)
    # edge tables + one-hot rings live in BUF[2]'s bytes (dead until conv1 mean)
    _tb32 = BUF[2].ap().bitcast(f32)       # [128, 8192] f32 view
    srct = _tb32[:, 0:EPC]                 # srch table
    srwt = _tb32[:, EPC:2 * EPC]           # srcw table
    dstt = _tb32[:, 2 * EPC:3 * EPC]       # dstp table
    _tb16 = BUF[2].ap().bitcast(fp16)      # [128, 16384] fp16 view
    CS0 = 3 * 2 * EPC                      # fp16-col offset after 24KB tables
    NRC, NRD, NRA = 4, 4, 2
    cs_ring = [_tb16[:, CS0 + i * 128: CS0 + (i + 1) * 128] for i in range(NRC)]
    OH0 = CS0 + NRC * 128
    ohd_ring = [_tb16[:, OH0 + i * 256: OH0 + (i + 1) * 256] for i in range(NRD)]
    T20 = OH0 + NRD * 256
    t2_ring = [_tb16[:, T20 + i * 256: T20 + (i + 1) * 256] for i in range(2)]
    oha_ring = [_tb16[:, T20 + (2 + i) * 256: T20 + (3 + i) * 256] for i in range(NRA)]
    _tb8 = BUF[2].ap().bitcast(fp8)        # [128, 32768] fp8 view
    E80 = 2 * (T20 + (2 + NRA) * 256)      # byte offset after fp16 rings
    NR8 = 3
    cs8_ring = [_tb8[:, E80 + i * 256: E80 + (i + 1) * 256] for i in range(NR8)]
    O80 = E80 + NR8 * 256
    ohd8_ring = [_tb8[:, O80 + i * 512: O80 + (i + 1) * 512] for i in range(NR8)]
    pint = [nc.alloc_sbuf_tensor(f"pint{i}", [128, 256], i32) for i in range(3)]
    iota = nc.alloc_sbuf_tensor("iota", [128, 256], fp16)
    ident = nc.alloc_sbuf_tensor("idents", [128, 128], f32)
    wcol = [nc.alloc_sbuf_tensor(f"wcolS{k}", [128, 1], fp16) for k in range(3)]
    wls = [nc.alloc_sbuf_tensor(f"wlS{k}", [F, H], fp16) for k in range(3)]
    wrs = [nc.alloc_sbuf_tensor(f"wrS{k}", [F, H], fp16) for k in range(3)]
    biass = [nc.alloc_sbuf_tensor(f"bS{k}", [H, 1], f32) for k in range(3)]
    eus = nc.alloc_sbuf_tensor("euS", [NT, 2 * G], f32)
    fus = nc.alloc_sbuf_tensor("fuS", [G, 2 * NT], f32)
    mcol = [nc.alloc_sbuf_tensor(f"mcol{k}", [128, NT], fp16) for k in range(2)]
    onesc = nc.alloc_sbuf_tensor("onesc", [128, 1], fp16)
    S = nc.alloc_sbuf_tensor("S", [G, 256], f32)
    m8 = nc.alloc_sbuf_tensor("m8", [G, 8], f32)
    rb = nc.alloc_sbuf_tensor("rb", [G, 8], f32)
    vv = nc.alloc_sbuf_tensor("vv", [G, 256], f32)
    wprev = nc.alloc_sbuf_tensor("wprev", [G, 256], f32)
    vnm = nc.alloc_sbuf_tensor("vnm", [128, NT], f32)
    strn = vnm
    xmaxb = nc.alloc_sbuf_tensor("xmaxb", [128, G], f32)
    za = nc.alloc_sbuf_tensor("za", [128, G], f32)
    zb = nc.alloc_sbuf_tensor("zb", [128, G], f32)
    uMk = nc.alloc_sbuf_tensor("uMk", [128, 256], f32)
    sraw = uMk.ap()[:, 0:NT]
    Mk = uMk.ap()[0:G, :]
    z1 = uMk.ap()[:, 0:G]
    z2 = uMk.ap()[0:64, G:2 * G]
    zo_t = nc.alloc_sbuf_tensor("zo", [T, G], f32)
    zo = zo_t.ap()
    mlpw = [nc.alloc_sbuf_tensor(n, s, f32) for n, s in
            [("l1waS", [128, 128]), ("l1wbS", [128, 128]), ("l2wS", [128, 64]),
             ("l3wS", [64, T]), ("l1bS", [128, 1]), ("l2bS", [64, 1]), ("l3bS", [T, 1])]]

    # ---------------- phase 0: loads & edge prep ----------------
    xnm = BUF[0]
    TCH = 16
    nc.sync.dma_start(iota.ap(), iota_d.ap())
    for eo in range(0, EPC, EPC // 4):
        nc.sync.dma_start(srct[:, eo:eo + EPC // 4], srch_d.ap()[:, eo:eo + EPC // 4])
        nc.sync.dma_start(srwt[:, eo:eo + EPC // 4], srcw_d.ap()[:, eo:eo + EPC // 4])
        nc.sync.dma_start(dstt[:, eo:eo + EPC // 4], dstp_d.ap()[:, eo:eo + EPC // 4])

    nc.sync.dma_start(ident.ap(), ident_d.ap())
    nc.sync.dma_start(eus.ap(), eu_d.ap())
    nc.sync.dma_start(fus.ap(), fu_d.ap())
    for k in range(3):
        nc.sync.dma_start(wcol[k].ap(), wcol_d[k].ap())
        nc.sync.dma_start(wls[k].ap(), wl[k].ap())
        nc.sync.dma_start(wrs[k].ap(), wr[k].ap())
        nc.sync.dma_start(biass[k].ap(), bias[k].ap())
    for s, d in zip(mlpw, [l1wa, l1wb, l2w, l3w, l1b, l2b, l3b]):
        nc.sync.dma_start(s.ap(), d.ap())
    nc.vector.memset(mcol[0].ap(), 1.0)
    nc.vector.memset(onesc.ap(), 1.0)
    for to in range(0, NT, TCH):
        nc.sync.dma_start(
            xnm.ap().rearrange("p (t f) -> p t f", t=NT)[:, to:to + TCH, :],
            xh.ap().rearrange("(t p) f -> p t f", p=128)[:, to:to + TCH, :])

    xfm = BUF[1]
    for to in range(0, NT, NT // 8):
        nc.sync.dma_start_transpose(
            xfm.ap().rearrange("q (t j) -> q t j", t=NT)[:, to:to + NT // 8, :],
            xnm.ap()[:, to * 128:(to + NT // 8) * 128])

    def _emit_agg(ga, src_buf, mc_in_t, dst_buf, pool_ag, pool_dg,
                  split_evac=False):
        ag = pool_ag.tile([128, 512], f32, tag="ag")
        first = True
        for kt in range(2):
            nt_i = 2 * ga + kt
            for h in range(2):
                lhs = A_all.ap()[:, ga * 512 + kt * 256 + h * 128:
                                 ga * 512 + kt * 256 + (h + 1) * 128]
                nc.tensor.matmul(ag[:, h * 128:(h + 1) * 128], lhs,
                                 src_buf.ap()[:, nt_i * 128:(nt_i + 1) * 128],
                                 start=first, stop=False)
                first = False
                nc.tensor.matmul(ag[:, 256 + h:257 + h], lhs,
                                 mc_in_t.ap()[:, nt_i:nt_i + 1],
                                 start=False, stop=(kt == 1 and h == 1))
        dg = pool_dg.tile([128, 2], f32, tag="dg")
        nc.vector.tensor_scalar(dg[:], ag[:, 256:258], 1.0, None, op0=Alu.max)
        nc.vector.reciprocal(dg[:], dg[:])
        for h in range(2):
            nt_o = 2 * ga + h
            dst_sl = dst_buf.ap()[:, nt_o * 128:(nt_o + 1) * 128]
            if split_evac:
                nc.vector.tensor_scalar(dst_sl, ag[:, h * 128:(h + 1) * 128],
                                        dg[:, h:h + 1], None, op0=Alu.mult)
            else:
                nc.scalar.activation(dst_sl, ag[:, h * 128:(h + 1) * 128],
                                     Act.Copy, scale=dg[:, h:h + 1])

    # ---------------- phase 1: adjacency build (packed src) ----------------
    if PHASES < 1:
        nc.vector.memset(zo, 0.0)
        with nc.allow_non_contiguous_dma(reason="t"):
            nc.sync.dma_start(out_d.ap().rearrange("g t -> t g"), zo)
        return

    def edge_col(g, kt):
        return g * 32 + kt

    scpX_cm = tc.tile_pool(name="scpX", bufs=1, space="PSUM")
    scpX = scpX_cm.__enter__()
    spsX = scpX.tile([128, NT], f32, tag="spsX")
    with tc.tile_pool(name="apsum", bufs=3, space="PSUM") as apsum, \
         tc.tile_pool(name="agg1", bufs=3, space="PSUM") as agg1p, \
         tc.tile_pool(name="dpe", bufs=1, space="PSUM") as dpep, \
         tc.tile_pool(name="deg1", bufs=3) as deg1p:
        ci = 0
        di = 0
        ptiles = {}

        def _unpack(gu):
            # hi = round((P-255.75)/512) (exact: (lo-255.75)/512 is strictly
            # inside (-0.5, 0.5)); ahi = fp16(hi); alo = P - 512*hi.
            # A_all layout: kt0 = even-parity source rows, kt1 = odd.
            Pu = ptiles.pop(gu)
            pi_t = pint[gu % 3].ap()
            nc.vector.tensor_scalar(pi_t, Pu[:], -255.75, 1.0 / 512.0,
                                    op0=Alu.add, op1=Alu.mult)
            nc.scalar.copy(A_all.ap()[:, gu * 512 + 256: gu * 512 + 512], pi_t)
            nc.vector.scalar_tensor_tensor(
                A_all.ap()[:, gu * 512: gu * 512 + 256], pi_t, -512.0, Pu[:],
                op0=Alu.mult, op1=Alu.add)

        for g in range(G):
            P = apsum.tile([128, 256], f32, tag="P")
            ptiles[g] = P
            # Pool path: fp8e5 packed one-hots feeding DoubleRow pairs that
            # OPEN the accumulation; the pool engine runs ahead of DVE.
            for pi in range(POOL_PAIRS):
                cs8 = cs8_ring[(g * POOL_PAIRS + pi) % NR8]
                ohd8 = ohd8_ring[(g * POOL_PAIRS + pi) % NR8]
                for half in range(2):
                    col = edge_col(g, 2 * pi + half)
                    nc.gpsimd.tensor_scalar(
                        cs8[:, half * 128:(half + 1) * 128], iota.ap()[:, 0:128],
                        srct[:, col:col + 1], srwt[:, col:col + 1],
                        op0=Alu.is_equal, op1=Alu.mult)
                    nc.gpsimd.tensor_scalar(
                        ohd8[:, half * 256:(half + 1) * 256], iota.ap(),
                        dstt[:, col:col + 1], None, op0=Alu.is_equal)
                c3 = cs8.rearrange("p (t n) -> p t n", t=2)
                d3 = ohd8.rearrange("p (t n) -> p t n", t=2)
                nc.tensor.matmul(P[:], c3, d3, start=(pi == 0), stop=False,
                                 perf_mode=PerfMode.DoubleRow)
            # remaining chunks: cs always on DVE; dst one-hot split across
            # Act (|d-iota| then relu(1-t)), Pool (fp16 single), and DVE.
            # singles order: pool-dst, dve, act LAST in the PSUM chain so the
            # act one-hots (slowest producer) have the whole graph interval.
            nsingle = 32 - 2 * POOL_PAIRS
            singles = ([("act", i) for i in range(ACT_CHUNKS)]
                       + [("pool", i) for i in range(POOL_DST)]
                       + [("dve", i) for i in range(DVE_CHUNKS)])
            for si, (eng, _i) in enumerate(singles):
                ch = 2 * POOL_PAIRS + si
                col = edge_col(g, ch)
                cs = cs_ring[ci % NRC]
                ci += 1
                if eng == "act":
                    t2 = t2_ring[_i % 2]
                    ohd = oha_ring[_i % NRA]
                    nc.scalar.activation(t2, iota.ap(), Act.Abs,
                                         bias=dstt[:, col:col + 1], scale=-1.0)
                    nc.scalar.activation(ohd, t2, Act.Relu, bias=1.0, scale=-1.0)
                elif eng == "pool":
                    ohd = ohd_ring[di % NRD]
                    di += 1
                    nc.gpsimd.tensor_scalar(ohd, iota.ap(),
                                            dstt[:, col:col + 1], None,
                                            op0=Alu.is_equal)
                else:
                    ohd = ohd_ring[di % NRD]
                    di += 1
                    nc.vector.tensor_scalar(ohd, iota.ap(),
                                            dstt[:, col:col + 1], None,
                                            op0=Alu.is_equal)
                nc.vector.tensor_scalar(cs, iota.ap()[:, 0:128],
                                        srct[:, col:col + 1], srwt[:, col:col + 1],
                                        op0=Alu.is_equal, op1=Alu.mult)
                nc.tensor.matmul(P[:], cs, ohd,
                                 start=(POOL_PAIRS == 0 and si == 0),
                                 stop=(si == nsingle - 1))
            # software pipeline: unpack graph g-2 (its PSUM is long done, so
            # nothing here blocks the engines' one-hot production for g/g+1),
            # and run conv1's aggregation a further graph behind.
            if g >= 2:
                _unpack(g - 2)
            if PHASES >= 2 and g >= 3:
                _emit_agg(g - 3, BUF[0], mcol[0], BUF[3], agg1p, deg1p)
            # conv1 early-start: BUF[2] bytes [256*to, 256*to+2048) hold one
            # table's entries for graphs [2*to mod 128, +16); once those
            # graphs' one-hots are built the bytes are dead, so the mean
            # transpose for that block can run on the idle DMA engines, and
            # the dense conv1 matmuls for its graphs on PE's spare capacity
            # (relu on Act only: DVE is saturated here).
            TR_SCHED = {20: (0,), 28: (32,), 36: (8, 40), 44: (64, 72),
                        52: (16, 48, 80)}
            DN_SCHED = {24: (0, 1), 32: (8, 9), 40: (2, 3, 10, 11),
                        48: (16, 17, 18, 19), 56: (4, 5, 12, 13, 20, 21)}
            if PHASES >= 2 and g in TR_SCHED:
                for to in TR_SCHED[g]:
                    nc.sync.dma_start_transpose(
                        BUF[2].ap().rearrange("q (t j) -> q t j", t=NT)
                        [:, to:to + NT // 16, :],
                        BUF[3].ap()[:, to * 128:(to + NT // 16) * 128])
            if PHASES >= 2 and g in DN_SCHED:
                for ch in DN_SCHED[g]:
                    dp = dpep.tile([128, 512], f32, tag="dpe")
                    sl = slice(ch * 512, (ch + 1) * 512)
                    nc.tensor.matmul(dp[:], wls[0].ap(),
                                     BUF[2].ap()[:, sl], start=True, stop=False)
                    nc.tensor.matmul(dp[:], wrs[0].ap(),
                                     BUF[1].ap()[:, sl], start=False, stop=True)
                    nc.scalar.activation(BUF[1].ap()[:, sl], dp[:],
                                         Act.Relu, bias=biass[0].ap())
                    for t in range(4 * ch, 4 * ch + 4):
                        nc.tensor.matmul(spsX[:, t:t + 1],
                                         BUF[1].ap()[:, t * 128:(t + 1) * 128],
                                         wcol[0].ap(), start=(t == 0), stop=False)
        for gu in (G - 2, G - 1):
            _unpack(gu)
        if PHASES >= 2:
            for ga in (G - 3, G - 2, G - 1):
                _emit_agg(ga, BUF[0], mcol[0], BUF[3], agg1p, deg1p)
            for to in (24, 56, 88, 96, 104, 112, 120):
                nc.sync.dma_start_transpose(
                    BUF[2].ap().rearrange("q (t j) -> q t j", t=NT)
                    [:, to:to + NT // 16, :],
                    BUF[3].ap()[:, to * 128:(to + NT // 16) * 128])

    # ---------------- phase 2: convs + pools ----------------
    if PHASES < 2:
        nc.vector.memset(zo, 0.0)
        with nc.allow_non_contiguous_dma(reason="t"):
            nc.sync.dma_start(out_d.ap().rearrange("g t -> t g"), zo)
        return
    cur_nm, cur_fm = BUF[0], BUF[1]
    free_bufs = [BUF[3], BUF[2]]

    NCONV = 3 if PHASES >= 9 else max(0, min(3, PHASES - 1))
    for k in range(NCONV):
        mean_nm, mean_fm = free_bufs
        new_fm = cur_fm          # in-place: dense output reuses cur_fm buffer
        new_nm = cur_nm
        mc_in = mcol[k % 2]
        mc_out = mcol[(k + 1) % 2]
        done_early = ({0, 1, 8, 9, 16, 17, 2, 3, 10, 11, 18, 19,
                       4, 5, 12, 13, 20, 21} if k == 0 else set())

        if k > 0:
            for to in range(0, NT, NT // 16):
                nc.sync.dma_start_transpose(
                    mean_fm.ap().rearrange("q (t j) -> q t j", t=NT)
                    [:, to:to + NT // 16, :],
                    mean_nm.ap()[:, to * 128:(to + NT // 16) * 128])

        NCH = NT * 128 // 512
        with tc.tile_pool(name=f"dp{k}", bufs=4, space="PSUM") as dpp, \
             tc.tile_pool(name=f"scr{k}", bufs=2, space="PSUM") as scp:
            sps_ = spsX if k == 0 else scp.tile([128, NT], f32, tag="scps")
            for ch in range(NCH):
                if ch in done_early:
                    continue
                dp = dpp.tile([128, 512], f32, tag="dp")
                sl = slice(ch * 512, (ch + 1) * 512)
                nc.tensor.matmul(dp[:], wls[k].ap(), mean_fm.ap()[:, sl], start=True, stop=False)
                nc.tensor.matmul(dp[:], wrs[k].ap(), cur_fm.ap()[:, sl], start=False, stop=True)
                if ch % 2:
                    nc.vector.tensor_scalar(new_fm.ap()[:, sl], dp[:],
                                            biass[k].ap(), 0.0,
                                            op0=Alu.add, op1=Alu.max)
                else:
                    nc.scalar.activation(new_fm.ap()[:, sl], dp[:], Act.Relu, bias=biass[k].ap())
                if k > 0 and ch % (NCH // 16) == NCH // 16 - 1:
                    to = (ch // (NCH // 16)) * (NT // 16)
                    nc.sync.dma_start_transpose(
                        new_nm.ap().rearrange("q (t j) -> q t j", t=NT)
                        [:, to:to + NT // 16, :],
                        new_fm.ap()[:, to * 128:(to + NT // 16) * 128])
            if k == 0:
                for to in range(0, NT, NT // 16):
                    nc.sync.dma_start_transpose(
                        new_nm.ap().rearrange("q (t j) -> q t j", t=NT)
                        [:, to:to + NT // 16, :],
                        new_fm.ap()[:, to * 128:(to + NT // 16) * 128])
            for t in range(NT):
                if k == 0 and (t // 4) in done_early:
                    continue
                nc.tensor.matmul(sps_[:, t:t + 1],
                                 new_fm.ap()[:, t * 128:(t + 1) * 128],
                                 wcol[k].ap(),
                                 start=(k > 0 and t == 0), stop=(t == NT - 1))
            nc.vector.tensor_copy(sraw, sps_[:])

        if k == 0:
            scpX_cm.__exit__(None, None, None)
        with tc.tile_pool(name=f"sas{k}", bufs=2, space="PSUM") as sas:
            pt = sas.tile([NT, 128], f32, tag="pt")
            nc.tensor.transpose(pt[:], sraw, ident.ap())
            nc.vector.tensor_copy(strn.ap(), pt[:])
            sp_ = sas.tile([G, 256], f32, tag="sp")
            for u in range(2):
                nc.tensor.matmul(sp_[:, u * 128:(u + 1) * 128],
                                 eus.ap()[:, u * G:(u + 1) * G], strn.ap(),
                                 start=(u == 0), stop=(u == 1))
            # consume the score PSUM directly: tanh on Act, negate(+mask) on DVE
            nc.scalar.activation(vv.ap(), sp_[:], Act.Tanh)
            tneg = S
            if k == 0:
                nc.vector.tensor_scalar_mul(tneg.ap(), sp_[:], -1.0)
            else:
                nc.vector.scalar_tensor_tensor(tneg.ap(), sp_[:], -1.0, wprev.ap(),
                                               op0=Alu.mult, op1=Alu.add)
        drop = DROPS[k]
        full, rem = drop // 8, drop % 8
        for r in range(full):
            nc.vector.max(m8.ap(), tneg.ap())
            nc.vector.match_replace(tneg.ap(), m8.ap(), tneg.ap(), -1e30)
        if rem:
            nc.vector.max(m8.ap(), tneg.ap())
            nc.vector.memset(rb.ap(), 1e30)
            nc.vector.tensor_copy(rb.ap()[:, 0:rem], m8.ap()[:, 0:rem])
            nc.vector.match_replace(tneg.ap(), rb.ap(), tneg.ap(), -1e30)
        nc.vector.tensor_scalar(Mk, tneg.ap(), -1e29, None, op0=Alu.is_gt)
        nc.vector.tensor_tensor(vv.ap(), vv.ap(), Mk, op=Alu.mult)
        nc.vector.tensor_scalar(wprev.ap(), Mk, 1.0, 1e30,
                                op0=Alu.subtract, op1=Alu.mult)

        with tc.tile_pool(name=f"mnm{k}", bufs=2, space="PSUM") as mnp:
            mn = mnp.tile([128, NT], f32, tag="mn")
            vn = mnp.tile([128, NT], f32, tag="vn")
            for u in range(2):
                st, sp2 = u == 0, u == 1
                nc.tensor.matmul(mn[:], Mk[:, u * 128:(u + 1) * 128],
                                 fus.ap()[:, u * NT:(u + 1) * NT], start=st, stop=sp2)
                nc.tensor.matmul(vn[:], vv.ap()[:, u * 128:(u + 1) * 128],
                                 fus.ap()[:, u * NT:(u + 1) * NT], start=st, stop=sp2)
            nc.vector.tensor_copy(mc_out.ap(), mn[:])
            nc.vector.tensor_copy(vnm.ap(), vn[:])

        nxt_mean = new_fm if k < 2 else None
        with tc.tile_pool(name=f"aggz{k}", bufs=4, space="PSUM") as aggzp, \
             tc.tile_pool(name=f"degz{k}", bufs=3) as degzp:
            for g in range(G):
                for t in (2 * g, 2 * g + 1):
                    sl_t = new_nm.ap()[:, t * 128:(t + 1) * 128]
                    if g % 3 == 2:
                        nc.gpsimd.tensor_scalar(sl_t, sl_t, vnm.ap()[:, t:t + 1],
                                                None, op0=Alu.mult)
                    else:
                        nc.vector.tensor_scalar(sl_t, sl_t, vnm.ap()[:, t:t + 1],
                                                None, op0=Alu.mult)
                if k < 2:
                    _emit_agg(g, new_nm, mcol[(k + 1) % 2], nxt_mean, aggzp,
                              degzp)

        new_fm2 = mean_nm
        for to in range(0, NT, NT // 16):
            nc.sync.dma_start_transpose(
                new_fm2.ap().rearrange("q (t j) -> q t j", t=NT)[:, to:to + NT // 16, :],
                new_nm.ap()[:, to * 128:(to + NT // 16) * 128])

        with tc.tile_pool(name=f"pool{k}", bufs=2, space="PSUM") as plp:
            # max-pool via pairwise halving tree: rounds 1-2 on Pool (idle in
            # phase 2), rest on DVE; scratch lives in mean_fm (dead after the
            # dense matmuls read it).
            scr = mean_fm.ap()
            src_v = new_fm2.ap().rearrange("q (g n) -> q g n", g=G)
            off = 0
            w = 128
            nc.vector.tensor_tensor(
                scr[:, off:off + G * w].rearrange("q (g n) -> q g n", g=G),
                src_v[:, :, 0:128], src_v[:, :, 128:256], op=Alu.max)
            while w > 1:
                prev, off = off, off + G * w
                w //= 2
                pv = scr[:, prev:prev + G * 2 * w].rearrange("q (g n) -> q g n", g=G)
                if w > 1:
                    nc.vector.tensor_tensor(
                        scr[:, off:off + G * w].rearrange("q (g n) -> q g n", g=G),
                        pv[:, :, 0:w], pv[:, :, w:2 * w], op=Alu.max)
                else:
                    nc.vector.tensor_tensor(xmaxb.ap(), pv[:, :, 0], pv[:, :, 1],
                                            op=Alu.max)
            sps = plp.tile([128, G], f32, tag="sps")
            for g in range(G):
                for kt in range(2):
                    nc.tensor.matmul(sps[:, g:g + 1],
                                     new_nm.ap()[:, (2 * g + kt) * 128:(2 * g + kt + 1) * 128],
                                     onesc.ap(), start=(g == 0 and kt == 0),
                                     stop=(g == G - 1 and kt == 1))
            if k == 0:
                nc.vector.tensor_copy(za.ap(), xmaxb.ap())
                nc.vector.tensor_scalar_mul(zb.ap(), sps[:], 1.0 / KS[k])
            else:
                nc.vector.tensor_tensor(za.ap(), za.ap(), xmaxb.ap(), op=Alu.add)
                nc.vector.scalar_tensor_tensor(zb.ap(), sps[:], 1.0 / KS[k], zb.ap(),
                                               op0=Alu.mult, op1=Alu.add)

        cur_nm, cur_fm = new_nm, new_fm2
        # next conv's mean lands where aggz wrote it (new_fm); the other free
        # buffer (mean_fm, used as max-tree scratch) becomes its transpose.
        free_bufs = [new_fm, mean_fm]

    # ---------------- phase 3: MLP ----------------
    if PHASES < 9:
        nc.vector.memset(zo, 0.0)
        with nc.allow_non_contiguous_dma(reason="t"):
            nc.sync.dma_start(out_d.ap().rearrange("g t -> t g"), zo)
        return
    with tc.tile_pool(name="mlp", bufs=1, space="PSUM") as mpp:
        p1 = mpp.tile([128, G], f32, tag="p1")
        nc.tensor.matmul(p1[:], mlpw[0].ap(), za.ap(), start=True, stop=False)
        nc.tensor.matmul(p1[:], mlpw[1].ap(), zb.ap(), start=False, stop=True)
        nc.scalar.activation(z1, p1[:], Act.Relu, bias=mlpw[4].ap())
        p2 = mpp.tile([64, G], f32, tag="p2")
        nc.tensor.matmul(p2[:], mlpw[2].ap(), z1, start=True, stop=True)
        nc.scalar.activation(z2, p2[:], Act.Relu, bias=mlpw[5].ap())
        p3 = mpp.tile([T, G], f32, tag="p3")
        nc.tensor.matmul(p3[:], mlpw[3].ap(), z2, start=True, stop=True)
        nc.vector.tensor_scalar(zo, p3[:], mlpw[6].ap(), None, op0=Alu.add)
    with nc.allow_non_contiguous_dma(reason="tiny [T,G] final output"):
        nc.sync.dma_start(out_d.ap().rearrange("g t -> t g"), zo)


# node permutation: pos(n) = (n>>1) + 128*(n&1); inv_perm[pos] = node
_INV_PERM = np.concatenate([2 * np.arange(128), 2 * np.arange(128) + 1]).astype(np.int64)


def prep_host_inputs(inputs, n_cores=N_CORES):
    bf = np.float16
    NT = 2 * G
    x = np.asarray(inputs["x"], np.float32)
    ei = np.asarray(inputs["edge_index"], np.int32)
    NNc, NEc = G * NPG, G * EPG

    consts = {}
    consts["iota256"] = np.tile(np.arange(256, dtype=np.float32)[None, :], (128, 1)).astype(np.float16)
    consts["ident"] = np.eye(128, dtype=np.float32)
    eu = np.zeros((NT, 2 * G), np.float32)
    fu = np.zeros((G, 2 * NT), np.float32)
    for u in range(2):
        for g in range(G):
            eu[2 * g + u, u * G + g] = 1.0
            fu[g, u * NT + 2 * g + u] = 1.0
    consts["eu"], consts["fu"] = eu, fu
    for k, nm in enumerate(["pool1_w", "pool2_w", "pool3_w"]):
        w = np.asarray(inputs[nm], np.float32)
        w = w / np.linalg.norm(w)
        consts[f"wcol{k}"] = w.reshape(128, 1).astype(bf)
    for k, nm in enumerate(["conv1", "conv2", "conv3"]):
        consts[f"w{k}l"] = np.ascontiguousarray(np.asarray(inputs[f"{nm}_Wl"], np.float32).T).astype(bf)
        consts[f"w{k}r"] = np.ascontiguousarray(np.asarray(inputs[f"{nm}_Wr"], np.float32).T).astype(bf)
        consts[f"b{k}"] = np.asarray(inputs[f"{nm}_b"], np.float32).reshape(H, 1)
    l1 = np.asarray(inputs["lin1_W"], np.float32).T
    consts["l1wa"] = np.ascontiguousarray(l1[0:128, :])
    consts["l1wb"] = np.ascontiguousarray(l1[128:256, :])
    consts["l2w"] = np.ascontiguousarray(np.asarray(inputs["lin2_W"], np.float32).T)
    consts["l3w"] = np.ascontiguousarray(np.asarray(inputs["lin3_W"], np.float32).T)
    consts["l1b"] = np.asarray(inputs["lin1_b"], np.float32).reshape(128, 1)
    consts["l2b"] = np.asarray(inputs["lin2_b"], np.float32).reshape(64, 1)
    consts["l3b"] = np.asarray(inputs["lin3_b"], np.float32).reshape(T, 1)

    def to_tbl(v):
        # edge e = g*4096 + ch*128 + p  ->  [p, g*32 + ch]
        return np.ascontiguousarray(
            np.transpose(v.reshape(G, 32, 128), (2, 0, 1)).reshape(128, NEc // 128)
        ).astype(np.float32)

    in_maps = []
    for c in range(n_cores):
        m = dict(consts)
        xc = x[c * NNc:(c + 1) * NNc].reshape(G, NPG, F)
        m["xh"] = np.ascontiguousarray(xc[:, _INV_PERM, :].reshape(NNc, F)).astype(np.float16)
        src = (ei[0, c * NEc:(c + 1) * NEc] & 255).astype(np.int64)
        dst = (ei[1, c * NEc:(c + 1) * NEc] & 255).astype(np.int64)
        m["srch"] = to_tbl(src >> 1)
        m["srcw"] = to_tbl(1 + 511 * (src & 1))
        m["dstp"] = to_tbl((dst >> 1) + 128 * (dst & 1))
        in_maps.append(m)
    return in_maps


_CACHE = {}


def _get_nc():
    if "nc" not in _CACHE:
        nc = bacc.Bacc("TRN2", target_bir_lowering=False, debug=False,
                       num_devices=N_CORES)
        with TileContext(nc) as tc:
            build_gnn(nc, tc)
        nc.compile()
        _CACHE["nc"] = nc
    return _CACHE["nc"]


def run_sharded(inputs, trace=False, **kw):
    nc = _get_nc()
    in_maps = prep_host_inputs(inputs)
    res = bass_utils.run_bass_kernel_spmd(
        nc, in_maps, core_ids=list(range(N_CORES)), trace=trace, **kw)
    out = np.concatenate([res.results[c]["out"] for c in range(N_CORES)], axis=0)
    return out.astype(np.float32), res


def kernel(**inputs):
    out, _ = run_sharded(inputs)
    return out
